# revision 20
# baseline (speedup 1.0000x reference)
"""Causal self-attention (B=4, T=2048, C=1024, H=16) on 8 TRN2 NeuronCores.

Sharding: core = 2*b + hg handles batch b and heads hg*8..hg*8+8 (hybrid
batch x tensor parallel). Each core computes QKV for its heads, causal
attention, and a partial output projection over its 512 y-columns.
Host sums the two partials per batch, transposes, and adds the combined
bias b_proj + w_proj @ b_v (the V-bias contribution is constant across
tokens because softmax rows sum to 1, so it is hoisted to the host).

Device-side layouts (t=token, c=embed, f=qkv feature, d=head dim):
  xT  [1024, 2048]  x[b].T                                (f32r)
  wqk [1024, 1024]  cols: 512 Q (pre-scaled by 1/8) then 512 K (f32r)
  wv  [1024, 512]                                         (f32r)
  wp  [512, 1024]   w_proj[:, my_cols].T                  (f32r)
  bqk [128, 8]      per f-tile bias columns (Q part /8)   (f32)
  tri [128, 128]    0 on k<=q else -1e30                  (f32)
Output: outT [1024, 2048] partial out^T (no bias).

Pipeline: segments qb = 0..3. Within segment qb the 4 head pairs run
as 2 sequential pair-pairs whose kt-iterations are interleaved two
abreast, so each stream's S/P@V matmuls (PE) cover the other stream's
exp latency (ACT). The p_t pool is 4 deep: with 3 buffers the exp of
iteration i WAR-waits on the P@V reads of iteration i-3, which on
hardware costs ~0.6us per iteration (~90us total).
  PE : QKV(ts qb+1), S^T + P@V for q-block qb, out-projection ts qb-.
  ACT: exp(S^T)                          (the attention bottleneck)
  DVE: QKV PSUM->SBUF copies, diag masks, softmax normalization
  POOL: partition_broadcast of 1/denominator rows

Attention per head-pair p (heads 2p, 2p+1 stacked on partitions 0:64 /
64:128 of QT[p], KT[p]):
  S^T[tk, tq] = K_block @ Q^T  (row-packed K=64 matmul pair)
  P = exp(S^T + trimask)  -> bf16  (no max subtraction; scores ~ N(0,1))
  [y_un^T; den] = V_aug^T @ P  (V_aug bf16 with a ones column, M=65)
  y^T = y_un^T * partition_broadcast(1/den)
Projection: out^T[o, t] = wp^T @ y^T accumulated over 4 c-chunks.
"""
from contextlib import ExitStack

import numpy as np

N_HEAD = 16
C = 1024
B = 4
T = 2048
D = 64
NCC = C // 128  # c chunks
NTT = T // 128  # t tiles
NTS = T // 512  # t spans / q blocks

_CACHE = {}


def _build_nc(reps=1, mode="full", pbufs=4, host_prologue=True, ycopy_norm=True, staggered=False):
    import concourse.mybir as mybir
    import concourse.tile as tile
    from concourse import bacc

    f32 = mybir.dt.float32
    bf16 = mybir.dt.bfloat16

    nc = bacc.Bacc()
    xT = nc.declare_dram_parameter("xT", [C, T], bf16, isOutput=False)
    wqk = nc.declare_dram_parameter("wqk", [C, 1024], bf16, isOutput=False)
    wv = nc.declare_dram_parameter("wv", [C, 512], bf16, isOutput=False)
    wp = nc.declare_dram_parameter("wp", [512, C], bf16, isOutput=False)
    bqk = nc.declare_dram_parameter("bqk", [128, 8], f32, isOutput=False)
    tri = nc.declare_dram_parameter("tri", [128, 128], f32, isOutput=False)
    outT = nc.declare_dram_parameter("outT", [C, T], bf16, isOutput=True)

    Exp = mybir.ActivationFunctionType.Exp

    with tile.TileContext(nc) as tc, ExitStack() as ctx:
        persist = ctx.enter_context(tc.tile_pool(name="persist", bufs=1))
        wpool = ctx.enter_context(tc.tile_pool(name="wpool", bufs=1))
        xpool = ctx.enter_context(tc.tile_pool(name="xpool", bufs=1))
        ppool = ctx.enter_context(tc.tile_pool(name="ppool", bufs=pbufs))
        npool = ctx.enter_context(tc.tile_pool(name="npool", bufs=2))
        opool = ctx.enter_context(tc.tile_pool(name="opool", bufs=2))
        pss = ctx.enter_context(tc.tile_pool(name="pss", bufs=2, space="PSUM"))
        psy = ctx.enter_context(tc.tile_pool(name="psy", bufs=1, space="PSUM"))

        bqk_sb = persist.tile([128, 8], f32)
        onecol = persist.tile([128, 8], bf16)
        nc.vector.memset(onecol, 1.0)
        tri2_sb = persist.tile([128, 2, 128], f32)

        QT = [persist.tile([128, T], bf16, tag=f"qt{p}", name=f"qt{p}") for p in range(4)]
        KT = [persist.tile([128, T], bf16, tag=f"kt{p}", name=f"kt{p}") for p in range(4)]
        V = [persist.tile([128, 8, 65], bf16, tag=f"v{tt}", name=f"v{tt}") for tt in range(NTT)]
        Y = [persist.tile([128, T], bf16, tag=f"y{p}", name=f"y{p}") for p in range(4)]
        # the ones column of V_aug (row-sum/denominator trick) is constant:
        # write it once instead of every rep inside v_group
        for tt in range(NTT):
            nc.vector.tensor_copy(
                out=V[tt][:, :, 64:65].rearrange("p h o -> p (h o)"),
                in_=onecol,
            )

        # ---- DMA priming order: first-needed data first ----
        wqk_sb = [None] * NCC
        wv_sb = [None] * NCC
        wqk_sb[0] = wpool.tile([128, 1024], bf16, tag="wqk0", name="wqk0")
        nc.sync.dma_start(out=wqk_sb[0], in_=wqk[0:128, :])
        nc.sync.dma_start(out=tri2_sb[:, 0, :], in_=tri[:, :])
        nc.sync.dma_start(out=tri2_sb[:, 1, :], in_=tri[:, :])
        nc.sync.dma_start(out=bqk_sb, in_=bqk[:, :])

        def load_x_full():
            """One contiguous 512KB DMA per c-chunk covering all of T."""
            xs = []
            for cc in range(NCC):
                t_ = xpool.tile([128, T], bf16, tag=f"x{cc}", name=f"x{cc}")
                nc.sync.dma_start(out=t_, in_=xT[cc * 128:(cc + 1) * 128, :])
                xs.append(t_)
            return xs

        xs0 = load_x_full()
        for cc in range(1, NCC):
            wqk_sb[cc] = wpool.tile([128, 1024], bf16, tag=f"wqk{cc}", name=f"wqk{cc}")
            nc.sync.dma_start(out=wqk_sb[cc], in_=wqk[cc * 128:(cc + 1) * 128, :])
        for cc in range(NCC):
            wv_sb[cc] = wpool.tile([128, 512], bf16, tag=f"wv{cc}", name=f"wv{cc}")
            nc.sync.dma_start(out=wv_sb[cc], in_=wv[cc * 128:(cc + 1) * 128, :])
        wp_sb = []
        for cc in range(4):
            t_ = wpool.tile([128, 1024], bf16, tag=f"wp{cc}", name=f"wp{cc}")
            nc.sync.dma_start(out=t_, in_=wp[cc * 128:(cc + 1) * 128, :])
            wp_sb.append(t_)

        def qk_group(ts, xs, mf):
            """One QT/KT projection matmul group (8 chained)."""
            pq = pss.tile([128, 512], f32, tag="s", name="pq", padded_shape=[128, 1024])
            for cc in range(NCC):
                nc.tensor.matmul(
                    pq,
                    wqk_sb[cc][:, mf * 128:(mf + 1) * 128],
                    xs[cc][:, ts * 512:(ts + 1) * 512],
                    start=(cc == 0),
                    stop=(cc == NCC - 1),
                )
            dst = QT[mf] if mf < 4 else KT[mf - 4]
            nc.vector.tensor_scalar_add(
                dst[:, ts * 512:(ts + 1) * 512], pq, bqk_sb[:, mf:mf + 1]
            )

        def v_group(ts, xs, tt4):
            """One V projection matmul group."""
            tt = ts * 4 + tt4
            pv = pss.tile([128, 512], f32, tag="s", name="pv", padded_shape=[128, 1024])
            for cc in range(NCC):
                nc.tensor.matmul(
                    pv,
                    xs[cc][:, tt * 128:(tt + 1) * 128],
                    wv_sb[cc],
                    start=(cc == 0),
                    stop=(cc == NCC - 1),
                )
            nc.vector.tensor_copy(
                out=V[tt][:, :, 0:64],
                in_=pv.rearrange("p (h d) -> p h d", h=8),
            )

        def qkv_groups(ts, xs):
            """All 12 matmul groups of the QKV projection for t-span ts."""
            for mf in range(8):
                yield lambda mf=mf: qk_group(ts, xs, mf)
            for tt4 in range(4):
                yield lambda tt4=tt4: v_group(ts, xs, tt4)

        def proj_group(ts2, mo):
            """One output-projection group: 2x 4-chained N=512 matmuls into a
            2-bank PSUM tile, one wide copy + one wide store."""
            po = pss.tile([128, 1024], f32, tag="s", name="po")
            for half in range(2):
                t0 = ts2 * 1024 + half * 512
                for cc in range(4):
                    nc.tensor.matmul(
                        po[:, half * 512:(half + 1) * 512],
                        wp_sb[cc][:, mo * 128:(mo + 1) * 128],
                        Y[cc][:, t0:t0 + 512],
                        start=(cc == 0),
                        stop=(cc == 3),
                    )
            ot = opool.tile([128, 1024], bf16, tag="ot", name="ot")
            nc.vector.tensor_copy(ot, po)
            nc.sync.dma_start(
                out=outT[mo * 128:(mo + 1) * 128, ts2 * 1024:(ts2 + 1) * 1024],
                in_=ot,
            )

        def proj_groups(ts2):
            for mo in range(8):
                yield lambda mo=mo: proj_group(ts2, mo)

        def attention_phase(qbs):
            """All pair-pair tasks of the listed segments, flat-pipelined:
            within a task the S matmuls run one kt ahead of the exps; at a
            task's last iteration the vacant prefetch slots host the NEXT
            task's S prologue, so ACT never drains at task boundaries."""
            def make_task(qb, pp):
                q0 = 512 * qb
                n_kt = 4 * qb + 4
                p1, p2 = 2 * pp, 2 * pp + 1

                def s_stage(p, kt):
                    s_t = pss.tile([128, 1024], f32, tag="s", name="s")
                    c_lo = max(0, 128 * kt - q0)
                    nc.tensor.matmul(
                        s_t[:, c_lo:512],
                        KT[p][0:64, 128 * kt:128 * kt + 128],
                        QT[p][0:64, q0 + c_lo:q0 + 512],
                        start=True, stop=True,
                    )
                    nc.tensor.matmul(
                        s_t[:, 512 + c_lo:1024],
                        KT[p][64:128, 128 * kt:128 * kt + 128],
                        QT[p][64:128, q0 + c_lo:q0 + 512],
                        start=True, stop=True,
                    )
                    d0 = 128 * kt - q0
                    if d0 >= 0:
                        nc.vector.tensor_add(
                            s_t.rearrange("p (h q) -> p h q", h=2)[:, :, d0:d0 + 128],
                            s_t.rearrange("p (h q) -> p h q", h=2)[:, :, d0:d0 + 128],
                            tri2_sb,
                        )
                    return s_t

                def exp_stage(kt, s_t):
                    p_t = ppool.tile([128, 2, 512], bf16, tag="p", name="p_t")
                    c_lo = max(0, 128 * kt - q0)
                    s3 = s_t.rearrange("p (h q) -> p h q", h=2)
                    if "noexp" not in mode:
                        nc.scalar.activation(
                            out=p_t[:, :, c_lo:], in_=s3[:, :, c_lo:], func=Exp
                        )
                    else:
                        nc.vector.memset(p_t[:, :, c_lo:], 0.5)
                    return p_t

                def pv_stage(p, kt, p_t, y_ps):
                    c_lo = max(0, 128 * kt - q0)
                    nc.tensor.matmul(
                        y_ps[:, c_lo:512], V[kt][:, 2 * p, :], p_t[:, 0, c_lo:],
                        start=(kt == 0), stop=(kt == n_kt - 1),
                    )
                    nc.tensor.matmul(
                        y_ps[:, 512 + c_lo:1024], V[kt][:, 2 * p + 1, :],
                        p_t[:, 1, c_lo:],
                        start=(kt == 0), stop=(kt == n_kt - 1),
                    )

                def norm_stage(p, y_ps):
                    if ycopy_norm:
                        # one fast copy frees the y PSUM banks; the recip/
                        # broadcast/mul chain then runs off SBUF at leisure
                        ysb = npool.tile([65, 1024], f32, tag="ysb", name="ysb")
                        nc.vector.tensor_copy(ysb, y_ps)
                        y_rd = ysb
                    else:
                        y_rd = y_ps
                    rb = npool.tile([64, 1024], f32, tag="rb", name="rb")
                    nc.vector.reciprocal(rb[0:1, :], y_rd[64:65, :])
                    nc.gpsimd.partition_broadcast(rb, rb[0:1, :])
                    nc.vector.tensor_mul(
                        Y[p][0:64, q0:q0 + 512], y_rd[0:64, 0:512], rb[:, 0:512]
                    )
                    nc.vector.tensor_mul(
                        Y[p][64:128, q0:q0 + 512], y_rd[0:64, 512:1024],
                        rb[:, 512:1024]
                    )

                return dict(n_kt=n_kt, p1=p1, p2=p2, s=s_stage, e=exp_stage,
                            pv=pv_stage, norm=norm_stage)

            tasks = [make_task(qb, pp) for qb in qbs for pp in range(2)]
            t0 = tasks[0]
            s1 = t0["s"](t0["p1"], 0)
            s2 = t0["s"](t0["p2"], 0)
            for i, t in enumerate(tasks):
                nxt = tasks[i + 1] if i + 1 < len(tasks) else None
                n_kt = t["n_kt"]
                y1 = psy.tile([65, 1024], f32, tag="y1", name="y1")
                y2 = psy.tile([65, 1024], f32, tag="y2", name="y2")
                for kt in range(n_kt):
                    p_t1 = t["e"](kt, s1)
                    p_t2 = t["e"](kt, s2)
                    if kt + 1 < n_kt:
                        s1n = t["s"](t["p1"], kt + 1)
                    elif nxt is not None and host_prologue:
                        s1n = nxt["s"](nxt["p1"], 0)
                    else:
                        s1n = None
                    t["pv"](t["p1"], kt, p_t1, y1)
                    if kt + 1 < n_kt:
                        s2n = t["s"](t["p2"], kt + 1)
                    elif nxt is not None and host_prologue:
                        s2n = nxt["s"](nxt["p2"], 0)
                    else:
                        s2n = None
                    t["pv"](t["p2"], kt, p_t2, y2)
                    s1, s2 = s1n, s2n
                t["norm"](t["p1"], y1)
                t["norm"](t["p2"], y2)
                if s1 is None and nxt is not None:
                    s1 = nxt["s"](nxt["p1"], 0)
                    s2 = nxt["s"](nxt["p2"], 0)

        # ---- phased body: QKV -> attention -> projection ----
        def body(xs0):
            xs = xs0 if xs0 is not None else load_x_full()
            for ts in range(NTS):
                for g in qkv_groups(ts, xs):
                    g()
            attention_phase(range(NTS))
            for ts2 in range(2):
                for g in proj_groups(ts2):
                    g()

        # ---- bench-only phase variants (never used by kernel()) ----
        if mode == "dma":
            otp = persist.tile([128, 512], bf16)
            nc.vector.memset(otp, 0.25)

            def body(xs0):
                load_x_full()
                for mo in range(8):
                    for ts in range(4):
                        nc.sync.dma_start(
                            out=outT[mo * 128:(mo + 1) * 128,
                                     ts * 512:(ts + 1) * 512],
                            in_=otp,
                        )

        if mode == "qkv":
            def body(xs0):
                xs = xs0 if xs0 is not None else load_x_full()
                for ts in range(NTS):
                    for g in qkv_groups(ts, xs):
                        g()

        if mode == "noproj":
            def body(xs0):
                xs = xs0 if xs0 is not None else load_x_full()
                for ts in range(NTS):
                    for g in qkv_groups(ts, xs):
                        g()
                attention_phase(range(NTS))

        if mode.startswith("attnonly"):
            for p in range(4):
                nc.vector.memset(QT[p], 0.01)
                nc.vector.memset(KT[p], 0.01)
            for tt in range(NTT):
                nc.vector.memset(V[tt], 1.0)

            def body(xs0):
                attention_phase(range(NTS))

        if mode == "tare":
            def body(xs0):
                pq = pss.tile([128, 512], f32, tag="s", name="pq", padded_shape=[128, 1024])
                nc.tensor.matmul(pq, wqk_sb[0][:, 0:128], xs0[0][:, 0:512], start=True, stop=True)
                nc.vector.tensor_scalar_add(QT[0][:, 0:512], pq, bqk_sb[:, 0:1])

        if reps == 1:
            body(xs0)
        else:
            ET = mybir.EngineType
            with tc.For_i(0, reps, 1,
                          hint_engines=(ET.PE, ET.DVE, ET.Activation, ET.SP, ET.Pool),
                          staggered_reset=staggered):
                body(None)

    nc.finalize()
    return nc


def _prep_core_inputs(x, w_attn, b_attn, w_proj, core):
    import ml_dtypes

    bf16 = ml_dtypes.bfloat16
    b, hg = core // 2, core % 2
    s = hg * 512
    xT = np.ascontiguousarray(x[b].T)
    wq = w_attn[s:s + 512] * 0.125
    wk = w_attn[1024 + s:1024 + s + 512]
    wqk = np.ascontiguousarray(np.concatenate([wq, wk], axis=0).T)
    wv = np.ascontiguousarray(w_attn[2048 + s:2048 + s + 512].T)
    wp = np.ascontiguousarray(w_proj[:, s:s + 512].T)
    bq = b_attn[s:s + 512] * 0.125
    bk = b_attn[1024 + s:1024 + s + 512]
    bqk = np.ascontiguousarray(np.concatenate([bq, bk]).reshape(8, 128).T)
    r = np.arange(128, dtype=np.int32)
    tri = np.where(r[:, None] <= r[None, :], 0.0, -1e30).astype(np.float32)
    return {
        "xT": xT.astype(bf16),
        "wqk": wqk.astype(bf16),
        "wv": wv.astype(bf16),
        "wp": wp.astype(bf16),
        "bqk": bqk.astype(np.float32),
        "tri": tri,
    }


def kernel(x, w_attn, b_attn, w_proj, b_proj):
    from concourse.bass_utils import run_bass_kernel_spmd

    x = np.asarray(x, dtype=np.float32)
    w_attn = np.asarray(w_attn, dtype=np.float32)
    b_attn = np.asarray(b_attn, dtype=np.float32)
    w_proj = np.asarray(w_proj, dtype=np.float32)
    b_proj = np.asarray(b_proj, dtype=np.float32)

    if "nc" not in _CACHE:
        _CACHE["nc"] = _build_nc()
    nc = _CACHE["nc"]

    in_maps = [
        _prep_core_inputs(x, w_attn, b_attn, w_proj, core) for core in range(8)
    ]
    res = run_bass_kernel_spmd(nc, in_maps, list(range(8)))

    # total bias: b_proj plus the (token-constant) V-bias contribution
    bias = b_proj + w_proj @ b_attn[2048:]
    out = np.empty((B, T, C), dtype=np.float32)
    for b in range(B):
        acc = (np.asarray(res.results[2 * b]["outT"], dtype=np.float32)
               + np.asarray(res.results[2 * b + 1]["outT"], dtype=np.float32))
        out[b] = acc.T + bias
    return out

